# revision 2
# baseline (speedup 1.0000x reference)
"""MIHash loss kernel for Trainium2 (8 NeuronCores, SPMD).

Math: loss = sum_i ent(pD_i) - prCp_i*ent(pDCp_i) - prCn_i*ent(pDCn_i)
where the 16-bin histograms come from triangular (hat) pulses of the soft
Hamming distance dist = (64 - phi@phi.T)/2, weighted by label-agreement
xp / xn.

Device strategy (per core, 1024 rows of the row-sorted problem):
  w = dist/4 - ... actually w = 8 - pp/8 in (0,16); hat centers at integers
  b=0..15. Using hat(x) = relu(x+1) - 2 relu(x) + relu(x-1):
      H[b] = R(b-1) - 2 R(b) + R(b+1),  R(c) = sum_j relu(w_ij - c)
  R(c) for c<=0 is linear (w>0): R(0)=T=sum w, R(-1)=T+count; R(16)=0.
  So only c=1..15 need real passes: one fused DVE/ACT op each
  (elementwise + accum_out per-partition sum).
  The xp-weighted histograms use the one-hot class structure: rows/cols
  sorted by label => same-class columns are a contiguous segment near the
  diagonal. A [128, WIN] band per 128-row block covers every segment; a
  host-built 0/1 mask M (diag excluded) gives wp = (wb+1)*M, and
  R_p(c) = sum relu(wp - (c+1)).
Host does O(N*nbins) pre/post-processing (sort, second differences,
entropies) in float64.
"""

import os
import numpy as np
import ml_dtypes

import concourse.bass as bass
import concourse.mybir as mybir
import concourse.tile as tile
from concourse import bacc
from concourse.bass_utils import run_bass_kernel_spmd

N = 8192
NBIT = 64
NCORES = 8
ROWS_PER_CORE = N // NCORES          # 1024
BLOCKS = ROWS_PER_CORE // 128        # 8
NBINS = 16
EPS = 1e-7
# Real (nonlinear) thresholds: w = 8 - phi_i.phi_j/8 concentrates in
# ~[6.0, 9.2], so R(c) is linear in T for c <= CLO and zero for c >= CHI.
# Validated at runtime via R(CLO) == T - CLO*N and R(CHI) == 0.
CLO = 6
CHI = 10
THRESH = list(range(CLO, CHI + 1))   # 5 device threshold passes

F32 = mybir.dt.float32
F16 = mybir.dt.float16
BF16 = mybir.dt.bfloat16

_PROGRAM_CACHE = {}

# threshold engine assignment: ACT (scalar), GPS (gpsimd), rest on DVE (vector)
ACT_THRESH = {8}
GPS_THRESH = set()                   # Pool can't run TensorScalar on this toolchain


def _build_program(pad: int):
    """One SPMD Bass program; per-core differences live in the input data."""
    win = 128 + 2 * pad              # mask window width per 128-row block
    bw = ROWS_PER_CORE + 2 * pad     # band width per core

    nc = bacc.Bacc(
        "TRN2", target_bir_lowering=False, debug=False, num_devices=NCORES
    )
    phiT_d = nc.dram_tensor("phiT", [NBIT, N], BF16, kind="ExternalInput")
    bandT_d = nc.dram_tensor("bandT", [NBIT, bw], BF16, kind="ExternalInput")
    mask_d = nc.dram_tensor("mmask", [BLOCKS, 128, win], F16, kind="ExternalInput")
    # per block: col c-1 / col 16+c-1 = half accums of threshold c (DVE/GPS),
    # or whole-pass accum in col c-1 (ACT); cols 15/31 = T halves
    rall_d = nc.dram_tensor("rall", [BLOCKS, 128, 32], F32, kind="ExternalOutput")
    # per block: col 0 = T_p ; cols 1..15 = R_p(c')
    rp_d = nc.dram_tensor("rp", [BLOCKS, 128, 16], F32, kind="ExternalOutput")

    sub = mybir.AluOpType.subtract
    mx = mybir.AluOpType.max
    add = mybir.AluOpType.add
    mult = mybir.AluOpType.mult
    relu = mybir.ActivationFunctionType.Relu
    ident = mybir.ActivationFunctionType.Identity

    with tile.TileContext(nc) as tc:
        with (
            tc.tile_pool(name="const", bufs=1) as constp,
            tc.tile_pool(name="big", bufs=2) as bigp,
            tc.tile_pool(name="scr", bufs=3) as scrp,
            tc.tile_pool(name="band", bufs=2) as bandp,
            tc.tile_pool(name="acc", bufs=1) as accp,
            tc.tile_pool(name="ps", bufs=2, space=bass.MemorySpace.PSUM) as psp,
            tc.tile_pool(name="psb", bufs=2, space=bass.MemorySpace.PSUM) as psbp,
        ):
            phiT = constp.tile([NBIT, N], BF16)
            bandT = constp.tile([NBIT, bw], BF16)
            nc.sync.dma_start(phiT[:], phiT_d[:])
            nc.sync.dma_start(bandT[:], bandT_d[:])

            # bias constants for ACT ops: col 0 = 8.0 (build), then -c per ACT thr
            act_cs = sorted(ACT_THRESH)
            biases = constp.tile([128, 1 + len(act_cs)], F32)
            nc.vector.memset(biases[:, 0:1], 8.0)
            bias8 = biases[:, 0:1]
            bias_col = {}
            for i, c in enumerate(act_cs):
                nc.vector.memset(biases[:, 1 + i : 2 + i], float(-c))
                bias_col[c] = biases[:, 1 + i : 2 + i]

            warm = constp.tile([128, 1], F32)
            nc.scalar.copy(warm[:], biases[:, 0:1])

            rall_s = accp.tile([128, BLOCKS * 32], F32)
            rp_s = accp.tile([128, BLOCKS * 16], F32)

            for blk in range(BLOCKS):
                own = bandT[:, pad + 128 * blk : pad + 128 * (blk + 1)]
                ra0 = blk * 32
                rp0 = blk * 16

                # ---- full side: w[128, N] fp16 then 15 threshold passes ----
                w = bigp.tile([128, N], F16, tag="w")
                for g in range(8):           # 8 psum groups of 1024
                    pp = psp.tile([128, 1024], F32, tag="pp")
                    for s in range(2):
                        nc.tensor.matmul(
                            pp[:, 512 * s : 512 * (s + 1)],
                            own,
                            phiT[:, 1024 * g + 512 * s : 1024 * g + 512 * (s + 1)],
                            start=True,
                            stop=True,
                        )
                    # w = 8 - pp/8 (always > 0); split build across DVE and ACT
                    if g < 2:
                        nc.vector.tensor_scalar(
                            w[:, 1024 * g : 1024 * (g + 1)],
                            pp[:], 64.0, -0.125, sub, mult,
                        )
                    else:
                        nc.scalar.activation(
                            w[:, 1024 * g : 1024 * (g + 1)],
                            pp[:], ident, bias=bias8, scale=-0.125,
                        )

                H = N // 2
                # T = sum_j fp16(w): needed so linear-region R's cancel the
                # fp16 quantization exactly in the second differences
                for h in range(2):
                    scr0 = scrp.tile([128, H], F16, tag="scr_v")
                    nc.vector.tensor_scalar(
                        scr0[:], w[:, h * H : (h + 1) * H], 0.0, None, mx, add,
                        accum_out=rall_s[:, ra0 + 15 + 16 * h : ra0 + 16 + 16 * h],
                    )
                for c in THRESH:
                    ci = c - CLO
                    if c in ACT_THRESH:
                        scr = scrp.tile([128, N], F16, tag="scr_a")
                        nc.scalar.activation(
                            scr[:], w[:], relu, bias=bias_col[c], scale=1.0,
                            accum_out=rall_s[:, ra0 + ci : ra0 + ci + 1],
                        )
                    else:
                        eng = nc.gpsimd if c in GPS_THRESH else nc.vector
                        for h in range(2):
                            scr = scrp.tile([128, H], F16, tag="scr_v")
                            eng.tensor_scalar(
                                scr[:], w[:, h * H : (h + 1) * H], float(c), None,
                                mx, add,
                                accum_out=rall_s[:, ra0 + 16 * h + ci : ra0 + 16 * h + ci + 1],
                            )

                # ---- band (same-class) side ----
                ppb = psbp.tile([128, win], F32, tag="ppb")
                off = 0
                while off < win:
                    cw = min(512, win - off)
                    nc.tensor.matmul(
                        ppb[:, off : off + cw],
                        own,
                        bandT[:, 128 * blk + off : 128 * blk + off + cw],
                        start=True,
                        stop=True,
                    )
                    off += cw
                wb = bandp.tile([128, win], F16, tag="wb")
                nc.vector.tensor_scalar(wb[:], ppb[:], 64.0, -0.125, sub, mult)
                mt = bandp.tile([128, win], F16, tag="mt")
                nc.sync.dma_start(mt[:], mask_d[blk])
                wp = bandp.tile([128, win], F16, tag="wp")
                # wp = (wb + 1) * M
                nc.vector.scalar_tensor_tensor(wp[:], wb[:], 1.0, mt[:], add, mult)
                # Tp + win = sum max(wp, 1)
                scrb0 = scrp.tile([128, win], F16, tag="scr_b")
                nc.vector.tensor_scalar(
                    scrb0[:], wp[:], 1.0, None, mx, add,
                    accum_out=rp_s[:, rp0 : rp0 + 1],
                )
                for c in THRESH:
                    ci = c - CLO
                    scrb = scrp.tile([128, win], F16, tag="scr_b")
                    nc.vector.tensor_scalar(
                        scrb[:], wp[:], float(c + 1), None, mx, add,
                        accum_out=rp_s[:, rp0 + 1 + ci : rp0 + 2 + ci],
                    )

            for blk in range(BLOCKS):
                nc.sync.dma_start(rall_d[blk], rall_s[:, blk * 32 : (blk + 1) * 32])
                nc.sync.dma_start(rp_d[blk], rp_s[:, blk * 16 : (blk + 1) * 16])

    nc.compile()
    return nc, win, bw


class _RangeViolation(Exception):
    pass


def _numpy_reference(u, y):
    """Exact fallback for non-one-hot y (never expected with the harness)."""
    u = u.astype(np.float64)
    y = y.astype(np.float64)
    n, nbits = u.shape
    aff = ((y @ y.T) > 0).astype(np.float64)
    np.fill_diagonal(aff, 0.0)
    xp = aff
    xn = 1.0 - aff
    phi = 2.0 / (1.0 + np.exp(-u)) - 1.0
    dist = (nbits - phi @ phi.T) * 0.5
    prCp = xp.sum(1) / (n - 1)
    prCn = 1.0 - prCp
    delta = nbits // NBINS
    pDCp = np.zeros((n, NBINS))
    pDCn = np.zeros((n, NBINS))
    for b in range(NBINS):
        mid = b * delta
        ind = (dist > mid - delta) & (dist <= mid + delta)
        pulse = np.where(ind, 1.0 - np.abs(dist - mid) / delta, 0.0)
        pDCp[:, b] = (pulse * xp).sum(1)
        pDCn[:, b] = (pulse * xn).sum(1)
    return _finish_loss(pDCp, pDCn, prCp, prCn, n)


def _finish_loss(pDCp, pDCn, prCp, prCn, n):
    pD = (pDCp + pDCn) / (n - 1)
    sum_p = pDCp.sum(1)
    sum_n = pDCn.sum(1)
    safe_p = np.where(sum_p > 0, sum_p, 1.0)
    safe_n = np.where(sum_n > 0, sum_n, 1.0)
    pDCp = np.where((sum_p > 0)[:, None], pDCp / safe_p[:, None], pDCp)
    pDCn = np.where((sum_n > 0)[:, None], pDCn / safe_n[:, None], pDCn)

    def ent(p):
        return -(p * np.log(p + EPS)).sum(1)

    loss = (ent(pD) - (prCp * ent(pDCp) + prCn * ent(pDCn))).sum()
    return np.array(loss, dtype=np.float32)


def kernel(u, y):
    u = np.ascontiguousarray(np.asarray(u), dtype=np.float32)
    y = np.asarray(y)
    assert u.shape == (N, NBIT)

    pos = y > 0
    if not (pos.sum(axis=1) == 1).all() or (y < 0).any():
        return _numpy_reference(u, np.asarray(y, np.float32))
    labels = pos.argmax(axis=1)

    perm = np.argsort(labels, kind="stable")
    labels_s = labels[perm]
    counts = np.bincount(labels_s, minlength=labels_s.max() + 1)
    starts = np.concatenate([[0], np.cumsum(counts)])
    seg_s = starts[labels_s]                 # per sorted row
    seg_e = starts[labels_s + 1]
    maxn = int(counts.max())

    pad = 256
    while maxn > pad + 1:
        pad += 128
    win = 128 + 2 * pad
    bw = ROWS_PER_CORE + 2 * pad

    key = pad
    if key not in _PROGRAM_CACHE:
        _PROGRAM_CACHE[key] = _build_program(pad)
    nc, win_, bw_ = _PROGRAM_CACHE[key]
    assert (win_, bw_) == (win, bw)

    phi = np.tanh(u / 2.0)
    phiT = np.ascontiguousarray(phi[perm].T.astype(ml_dtypes.bfloat16))
    phi64 = phiT.T.astype(np.float64)                    # sorted rows, bf16 values
    s_all = phi64.sum(axis=0)                            # [64]
    T_host = 8.0 * N - (phi64 @ s_all) / 8.0             # [N] sum_j w_ij (incl diag)
    ncls = len(counts)
    cls_sums = np.zeros((ncls, NBIT))
    np.add.at(cls_sums, labels_s, phi64)
    nseg = (seg_e - seg_s).astype(np.float64)
    Tp_host = (
        8.0 * (nseg - 1.0)
        - ((phi64 * (cls_sums[labels_s] - phi64)).sum(axis=1)) / 8.0
    )

    in_maps = []
    for core in range(NCORES):
        off = core * ROWS_PER_CORE
        lo = off - pad
        band = np.zeros((NBIT, bw), dtype=ml_dtypes.bfloat16)
        c0 = max(0, lo)
        c1 = min(N, off + ROWS_PER_CORE + pad)
        band[:, c0 - lo : c1 - lo] = phiT[:, c0:c1]

        mm = np.zeros((BLOCKS, 128, win), dtype=np.float16)
        for blk in range(BLOCKS):
            win0 = off + 128 * blk - pad     # global col of window x=0
            rows = np.arange(off + 128 * blk, off + 128 * (blk + 1))
            xs = seg_s[rows] - win0
            xe = seg_e[rows] - win0
            assert (xs >= 0).all() and (xe <= win).all(), "segment outside window"
            idx = np.arange(win)[None, :]
            mm[blk] = ((idx >= xs[:, None]) & (idx < xe[:, None])).astype(np.float16)
            mm[blk, np.arange(128), rows - win0] = 0.0   # exclude diagonal
        in_maps.append({"phiT": phiT, "bandT": band, "mmask": mm})

    try:
        return _postprocess_and_loss(nc, in_maps, seg_s, seg_e, pad, T_host, Tp_host)
    except _RangeViolation:
        return _numpy_reference(u, np.asarray(y, np.float32))


_LAST_RESULTS = None


def _postprocess_and_loss(nc, in_maps, seg_s, seg_e, pad, T_host, Tp_host):
    global _LAST_RESULTS
    res = run_bass_kernel_spmd(nc, in_maps, list(range(NCORES)))
    _LAST_RESULTS = res
    if os.environ.get("KERNEL_PROFILE", "0") == "1":
        import time as _time

        for trial in range(3):
            t0 = _time.perf_counter()
            run_bass_kernel_spmd(nc, in_maps, list(range(NCORES)))
            dt = _time.perf_counter() - t0
            print(f"exec wall trial {trial}: {dt*1e9:.0f} ns")

    # ---- host postprocessing (float64) ----
    S_all = float(N)
    pDCp = np.zeros((N, NBINS))
    pDCn = np.zeros((N, NBINS))
    Sp_all = np.zeros(N)
    for core in range(NCORES):
        out = res.results[core]
        rall = out["rall"].astype(np.float64)      # [8, 128, 24]
        rp = out["rp"].astype(np.float64)          # [8, 128, 16]
        off = core * ROWS_PER_CORE
        rows = np.arange(off, off + ROWS_PER_CORE)
        Sp = (seg_e[rows] - seg_s[rows] - 1).astype(np.float64)  # n_l - 1
        Sp_all[rows] = Sp

        T = (rall[:, :, 15] + rall[:, :, 31]).reshape(-1)
        if np.abs(T - T_host[rows]).max() > 50.0:
            raise _RangeViolation()
        # device thresholds c in THRESH; build full R(-1..16) with linear/zero fill
        Rdev = np.empty((ROWS_PER_CORE, len(THRESH)))
        for c in THRESH:
            ci = c - CLO
            if c in ACT_THRESH:
                Rdev[:, ci] = rall[:, :, ci].reshape(-1)
            else:
                Rdev[:, ci] = (
                    rall[:, :, ci] + rall[:, :, 16 + ci]
                ).reshape(-1) - float(N) * c
        # runtime validation of the w-range assumption
        if (
            np.abs(Rdev[:, 0] - (T - CLO * N)).max() > 50.0
            or np.abs(Rdev[:, -1]).max() > 50.0
        ):
            raise _RangeViolation()
        R = np.zeros((ROWS_PER_CORE, 18))        # columns = c = -1 .. 16
        for c in range(-1, CLO + 1):
            R[:, c + 1] = T - float(c) * N       # linear region (w > CLO)
        for c in THRESH:
            R[:, c + 1] = Rdev[:, c - CLO]
        # c > CHI: zero (w < CHI)
        H_all = R[:, 0:16] - 2.0 * R[:, 1:17] + R[:, 2:18]
        H_all[:, :CLO] = 0.0
        H_all[:, CHI + 1 :] = 0.0

        win = 128 + 2 * pad
        Tp = rp[:, :, 0].reshape(-1) - win
        if np.abs(Tp - Tp_host[rows]).max() > 50.0:
            raise _RangeViolation()
        Rpdev = np.empty((ROWS_PER_CORE, len(THRESH)))
        for c in THRESH:
            ci = c - CLO
            Rpdev[:, ci] = rp[:, :, 1 + ci].reshape(-1) - float(win) * (c + 1)
        if (
            np.abs(Rpdev[:, 0] - (Tp - CLO * Sp)).max() > 50.0
            or np.abs(Rpdev[:, -1]).max() > 50.0
        ):
            raise _RangeViolation()
        Rp = np.zeros((ROWS_PER_CORE, 18))
        for c in range(-1, CLO + 1):
            Rp[:, c + 1] = Tp - float(c) * Sp
        for c in THRESH:
            Rp[:, c + 1] = Rpdev[:, c - CLO]
        H_p = Rp[:, 0:16] - 2.0 * Rp[:, 1:17] + Rp[:, 2:18]
        H_p[:, :CLO] = 0.0
        H_p[:, CHI + 1 :] = 0.0

        H_all = np.maximum(H_all, 0.0)
        H_p = np.maximum(H_p, 0.0)
        H_n = np.maximum(H_all - H_p, 0.0)
        pDCp[rows] = H_p
        pDCn[rows] = H_n

    prCp = Sp_all / (N - 1)
    prCn = 1.0 - prCp
    return _finish_loss(pDCp, pDCn, prCp, prCn, N)



# revision 8
# speedup vs baseline: 2.1472x; 2.1472x over previous
"""MIHash loss kernel for Trainium2 (8 NeuronCores, SPMD) — v2.

Math: loss = sum_i ent(pD_i) - prCp_i*ent(pDCp_i) - prCn_i*ent(pDCn_i),
histograms from triangular pulses of w = dist/4 = 8 - phi_i.phi_j/8,
hat(w-b) for b = 0..15.  With off-diagonal w in (6,10) (validated), the
16 bins derive from R(c) = sum_j relu(w_ij - c) at c = 7, 8, 9 plus the
exact row sum T (host, fp64) via H[b] = R(b-1) - 2R(b) + R(b+1) with
linear fills (R(c) = T - cN for c <= 6) and zeros (c >= 10).

Device work per core (1024 rows of the label-sorted problem, 8 blocks
of 128): per 2048-col PSUM group, matmul (fp16 phi) then three
reduction taps read PSUM directly:
  ACT: relu((8-c) - p/8) + accum  -> R-part
  DVE: (p*0.125) min (8-c) + accum -> S-part, R-part = (8-c)*2048 - S
The same-class (xp) side uses a [128, win] band around the diagonal of
the sorted order: q = p/8 + A (A = 0 on own segment minus diagonal,
1000 elsewhere), then 3 DVE taps sum min(q, 8-c); R_p(c) = (8-c)*win - S.

Host (fp64): sort by label, T / Tp / diagonal handled exactly (no range
assumption on the diagonal), validation of the off-diagonal range from
R(7), R(9) against host expectations (fallback to numpy on violation),
second differences, entropies.
"""

import os
import numpy as np

import concourse.bass as bass
import concourse.mybir as mybir
import concourse.tile as tile
from concourse import bacc
from concourse.bass_utils import run_bass_kernel_spmd

N = 8192
NBIT = 64
NCORES = 8
ROWS_PER_CORE = N // NCORES          # 1024
BLOCKS = ROWS_PER_CORE // 128        # 8
NBINS = 16
EPS = 1e-7

GW = 2048                            # psum group width
NG = N // GW                         # 4 groups per block
TAPS = (7, 8, 9)                     # thresholds; a = 8 - c

# Engine per (tap_idx, group): 'A' = ACT (relu+accum), 'D' = DVE (min+accum).
# 7 ACT / 5 DVE; band (3 taps + q build) rides on DVE.
ASSIGN = {
    (0, 0): "A", (0, 1): "A", (0, 2): "A", (0, 3): "A",   # c=7
    (1, 0): "A", (1, 1): "D", (1, 2): "A", (1, 3): "D",   # c=8
    (2, 0): "D", (2, 1): "A", (2, 2): "D", (2, 3): "D",   # c=9
}

F32 = mybir.dt.float32
F16 = mybir.dt.float16

BIG = 1000.0                         # band mask offset for excluded columns

_PROGRAM_CACHE = {}


def _build_program(pad: int):
    win = 128 + 2 * pad              # band window width per 128-row block
    bw = ROWS_PER_CORE + 2 * pad     # per-core band width

    nc = bacc.Bacc(
        "TRN2", target_bir_lowering=False, debug=False, num_devices=NCORES
    )
    phiT_d = nc.dram_tensor("phiT", [NBIT, N], F16, kind="ExternalInput")
    bandT_d = nc.dram_tensor("bandT", [NBIT, bw], F16, kind="ExternalInput")
    amask_d = nc.dram_tensor("amask", [BLOCKS, 128, win], F16, kind="ExternalInput")
    # per block 16 f32 cols: [c7 g0..g3, c8 g0..g3, c9 g0..g3, band c7,c8,c9, 0]
    racc_d = nc.dram_tensor("racc", [BLOCKS, 128, 16], F32, kind="ExternalOutput")

    mn = mybir.AluOpType.min
    add = mybir.AluOpType.add
    mult = mybir.AluOpType.mult
    relu = mybir.ActivationFunctionType.Relu

    with tile.TileContext(nc) as tc:
        with (
            tc.tile_pool(name="const", bufs=1) as constp,
            tc.tile_pool(name="ascr", bufs=2) as ascrp,
            tc.tile_pool(name="dscr", bufs=2) as dscrp,
            tc.tile_pool(name="bq", bufs=2) as bqp,
            tc.tile_pool(name="acc", bufs=1) as accp,
            tc.tile_pool(name="ps", bufs=2, space=bass.MemorySpace.PSUM) as psp,
        ):
            phiT = constp.tile([NBIT, N], F16)
            bandT = constp.tile([NBIT, bw], F16)
            amask = constp.tile([128, BLOCKS * win], F16)
            nc.sync.dma_start(phiT[:], phiT_d[:])
            nc.sync.dma_start(bandT[:], bandT_d[:])
            for b in range(BLOCKS):
                nc.sync.dma_start(amask[:, b * win : (b + 1) * win], amask_d[b])

            racc_s = accp.tile([128, BLOCKS * 16], F32)
            nc.vector.memset(racc_s[:], 0.0)

            # per-tap ACT bias columns (const AP registry lacks -1.0)
            biases = constp.tile([128, len(TAPS)], F32)
            bias_col = {}
            for ti, c in enumerate(TAPS):
                nc.vector.memset(biases[:, ti : ti + 1], float(8 - c))
                bias_col[c] = biases[:, ti : ti + 1]

            # warm the ACT function table early (overlaps input DMA)
            warm = constp.tile([128, 1], F32)
            nc.vector.memset(warm[:], 0.0)
            wsc = constp.tile([128, 1], F32)
            nc.scalar.activation(wsc[:], warm[:], relu, bias=bias_col[8], scale=1.0)

            for blk in range(BLOCKS):
                own = bandT[:, pad + 128 * blk : pad + 128 * (blk + 1)]
                r0 = blk * 16

                for g in range(NG):
                    pp = psp.tile([128, GW], F32, tag="pp")
                    for s in range(GW // 512):
                        nc.tensor.matmul(
                            pp[:, 512 * s : 512 * (s + 1)],
                            own,
                            phiT[:, GW * g + 512 * s : GW * g + 512 * (s + 1)],
                            start=True,
                            stop=True,
                        )
                    for ti, c in enumerate(TAPS):
                        a = float(8 - c)
                        col = r0 + 4 * ti + g
                        if ASSIGN[(ti, g)] == "A":
                            scr = ascrp.tile([128, GW], F32, tag="as")
                            nc.scalar.activation(
                                scr[:], pp[:], relu, bias=bias_col[c], scale=-0.125,
                                accum_out=racc_s[:, col : col + 1],
                            )
                        else:
                            # out = min(pp, 8a); accum = sum  -> 8*sum(min(t,a))
                            scr = dscrp.tile([128, GW], F32, tag="ds")
                            nc.vector.tensor_scalar(
                                scr[:], pp[:], 8.0 * a, None, mn, add,
                                accum_out=racc_s[:, col : col + 1],
                            )

                # band: matmul window into a rotation slot, mask-add, 3 taps
                ppb = psp.tile([128, GW], F32, tag="pp")
                off = 0
                while off < win:
                    cw = min(512, win - off)
                    nc.tensor.matmul(
                        ppb[:, off : off + cw],
                        own,
                        bandT[:, 128 * blk + off : 128 * blk + off + cw],
                        start=True,
                        stop=True,
                    )
                    off += cw
                q = bqp.tile([128, win], F32, tag="q")
                nc.vector.scalar_tensor_tensor(
                    q[:], ppb[:, 0:win], 0.125,
                    amask[:, blk * win : (blk + 1) * win], mult, add,
                )
                for ti, c in enumerate(TAPS):
                    a = float(8 - c)
                    scr = bqp.tile([128, win], F32, tag="bs")
                    nc.vector.tensor_scalar(
                        scr[:], q[:], a, None, mn, add,
                        accum_out=racc_s[:, r0 + 12 + ti : r0 + 13 + ti],
                    )

            for blk in range(BLOCKS):
                nc.sync.dma_start(racc_d[blk], racc_s[:, blk * 16 : (blk + 1) * 16])

    nc.compile()
    return nc, win, bw


class _RangeViolation(Exception):
    pass


def _numpy_reference(u, y):
    """Exact fp64 fallback (non-one-hot y or off-diagonal range violation)."""
    u = u.astype(np.float64)
    y = y.astype(np.float64)
    n, nbits = u.shape
    aff = ((y @ y.T) > 0).astype(np.float64)
    np.fill_diagonal(aff, 0.0)
    xp = aff
    xn = 1.0 - aff
    phi = 2.0 / (1.0 + np.exp(-u)) - 1.0
    dist = (nbits - phi @ phi.T) * 0.5
    prCp = xp.sum(1) / (n - 1)
    prCn = 1.0 - prCp
    delta = nbits // NBINS
    pDCp = np.zeros((n, NBINS))
    pDCn = np.zeros((n, NBINS))
    for b in range(NBINS):
        mid = b * delta
        ind = (dist > mid - delta) & (dist <= mid + delta)
        pulse = np.where(ind, 1.0 - np.abs(dist - mid) / delta, 0.0)
        pDCp[:, b] = (pulse * xp).sum(1)
        pDCn[:, b] = (pulse * xn).sum(1)
    return _finish_loss(pDCp, pDCn, prCp, prCn, n)


def _finish_loss(pDCp, pDCn, prCp, prCn, n):
    pD = (pDCp + pDCn) / (n - 1)
    sum_p = pDCp.sum(1)
    sum_n = pDCn.sum(1)
    safe_p = np.where(sum_p > 0, sum_p, 1.0)
    safe_n = np.where(sum_n > 0, sum_n, 1.0)
    pDCp = np.where((sum_p > 0)[:, None], pDCp / safe_p[:, None], pDCp)
    pDCn = np.where((sum_n > 0)[:, None], pDCn / safe_n[:, None], pDCn)

    def ent(p):
        return -(p * np.log(p + EPS)).sum(1)

    loss = (ent(pD) - (prCp * ent(pDCp) + prCn * ent(pDCn))).sum()
    return np.array(loss, dtype=np.float32)


def _hat(x):
    return np.maximum(0.0, 1.0 - np.abs(x))


_LAST_RESULTS = None


def kernel(u, y):
    u = np.ascontiguousarray(np.asarray(u), dtype=np.float32)
    y = np.asarray(y)
    assert u.shape == (N, NBIT)

    pos = y > 0
    if not (pos.sum(axis=1) == 1).all() or (y < 0).any():
        return _numpy_reference(u, np.asarray(y, np.float32))
    labels = pos.argmax(axis=1)

    perm = np.argsort(labels, kind="stable")
    labels_s = labels[perm]
    counts = np.bincount(labels_s, minlength=labels_s.max() + 1)
    starts = np.concatenate([[0], np.cumsum(counts)])
    seg_s = starts[labels_s]
    seg_e = starts[labels_s + 1]
    maxn = int(counts.max())

    pad = 128
    while maxn - 1 > pad:
        pad += 128
    win = 128 + 2 * pad
    bw = ROWS_PER_CORE + 2 * pad

    if pad not in _PROGRAM_CACHE:
        _PROGRAM_CACHE[pad] = _build_program(pad)
    nc, win_, bw_ = _PROGRAM_CACHE[pad]
    assert (win_, bw_) == (win, bw)

    phi16 = np.tanh(u / 2.0).astype(np.float16)
    phiT = np.ascontiguousarray(phi16[perm].T)           # [64, N] f16, sorted
    phi64 = phiT.T.astype(np.float64)

    # host-exact sums (same f16 phi values the device sees)
    s_all = phi64.sum(axis=0)
    t_row = (phi64 @ s_all) / 8.0                        # sum_j t_ij incl diag
    t_diag = (phi64 * phi64).sum(axis=1) / 8.0
    w_diag = 8.0 - t_diag
    T_all = 8.0 * N - t_row                              # sum_j w_ij incl diag

    ncls = len(counts)
    cls_sums = np.zeros((ncls, NBIT))
    np.add.at(cls_sums, labels_s, phi64)
    nseg = (seg_e - seg_s).astype(np.float64)            # class size incl self
    Sp = nseg - 1.0
    Tp = 8.0 * Sp - ((phi64 * (cls_sums[labels_s] - phi64)).sum(axis=1)) / 8.0

    in_maps = []
    for core in range(NCORES):
        off = core * ROWS_PER_CORE
        lo = off - pad
        band = np.zeros((NBIT, bw), dtype=np.float16)
        c0 = max(0, lo)
        c1 = min(N, off + ROWS_PER_CORE + pad)
        band[:, c0 - lo : c1 - lo] = phiT[:, c0:c1]

        am = np.full((BLOCKS, 128, win), BIG, dtype=np.float16)
        idx = np.arange(win)[None, :]
        for blk in range(BLOCKS):
            w0 = off + 128 * blk - pad                   # global col of window x=0
            rows = np.arange(off + 128 * blk, off + 128 * (blk + 1))
            xs = seg_s[rows] - w0
            xe = seg_e[rows] - w0
            assert (xs >= 0).all() and (xe <= win).all(), "segment outside window"
            inside = (idx >= xs[:, None]) & (idx < xe[:, None])
            am[blk][inside] = 0.0
            am[blk, np.arange(128), rows - w0] = BIG     # exclude diagonal
        in_maps.append({"phiT": phiT, "bandT": band, "amask": am})

    try:
        return _postprocess_and_loss(
            nc, in_maps, seg_s, seg_e, pad, T_all, Tp, Sp, w_diag
        )
    except _RangeViolation:
        return _numpy_reference(u, np.asarray(y, np.float32))


def _postprocess_and_loss(nc, in_maps, seg_s, seg_e, pad, T_all, Tp, Sp, w_diag):
    global _LAST_RESULTS
    res = run_bass_kernel_spmd(nc, in_maps, list(range(NCORES)))
    _LAST_RESULTS = res

    win = 128 + 2 * pad
    pDCp = np.zeros((N, NBINS))
    pDCn = np.zeros((N, NBINS))
    for core in range(NCORES):
        out = res.results[core]
        racc = out["racc"].astype(np.float64)            # [8, 128, 16]
        off = core * ROWS_PER_CORE
        rows = np.arange(off, off + ROWS_PER_CORE)

        # full side: R_all(c) from mixed-engine group parts
        R_all = np.zeros((ROWS_PER_CORE, 3))
        for ti, c in enumerate(TAPS):
            a = float(8 - c)
            acc = np.zeros(ROWS_PER_CORE)
            for g in range(NG):
                part = racc[:, :, 4 * ti + g].reshape(-1)
                if ASSIGN[(ti, g)] == "A":
                    acc += part                          # relu sums directly
                else:
                    acc += a * GW - part / 8.0           # (8-c)*GW - sum(min(t,a))
            R_all[:, ti] = acc

        wd = w_diag[rows]
        Td = T_all[rows]
        # validation: off-diagonal w must lie in (6, 10)
        L7 = R_all[:, 0] + 7.0 * N - Td                  # sum relu(7-w) incl diag
        exc7 = L7 - np.maximum(7.0 - wd, 0.0)
        R9p = R_all[:, 2] - np.maximum(wd - 9.0, 0.0)    # off-diag sum relu(w-9)
        if (exc7 > 0.5).any() or (R9p > 0.5).any() or (exc7 < -0.5).any():
            raise _RangeViolation()

        # off-diagonal R'(c), linear fills, second differences
        Rp_ = np.zeros((ROWS_PER_CORE, 18))              # c = -1 .. 16
        Tdp = Td - wd                                    # off-diag row sum
        for c in range(-1, 7):
            Rp_[:, c + 1] = Tdp - float(c) * (N - 1)
        for ti, c in enumerate(TAPS):
            Rp_[:, c + 1] = R_all[:, ti] - np.maximum(wd - c, 0.0)
        H_all = Rp_[:, 0:16] - 2.0 * Rp_[:, 1:17] + Rp_[:, 2:18]
        H_all[:, :6] = 0.0
        H_all[:, 11:] = 0.0
        H_all = np.maximum(H_all, 0.0)

        # band side: R_p(c) = (8-c)*win - S_band
        Rb = np.zeros((ROWS_PER_CORE, 18))
        Spr = Sp[rows]
        Tpr = Tp[rows]
        for c in range(-1, 7):
            Rb[:, c + 1] = Tpr - float(c) * Spr
        for ti, c in enumerate(TAPS):
            a = float(8 - c)
            Rb[:, c + 1] = a * win - racc[:, :, 12 + ti].reshape(-1)
        H_p = Rb[:, 0:16] - 2.0 * Rb[:, 1:17] + Rb[:, 2:18]
        H_p[:, :6] = 0.0
        H_p[:, 11:] = 0.0
        H_p = np.maximum(H_p, 0.0)

        H_n = np.maximum(H_all - H_p, 0.0)
        # diagonal contributes to the xn histogram (xn_ii = 1)
        bins = np.arange(NBINS)[None, :]
        H_n += _hat(wd[:, None] - bins)
        pDCp[rows] = H_p
        pDCn[rows] = H_n

    prCp = Sp / (N - 1)
    prCn = 1.0 - prCp
    return _finish_loss(pDCp, pDCn, prCp, prCn, N)


# revision 10
# speedup vs baseline: 2.3829x; 1.1098x over previous
"""MIHash loss kernel for Trainium2 (8 NeuronCores, SPMD) — v2.

Math: loss = sum_i ent(pD_i) - prCp_i*ent(pDCp_i) - prCn_i*ent(pDCn_i),
histograms from triangular pulses of w = dist/4 = 8 - phi_i.phi_j/8,
hat(w-b) for b = 0..15.  With off-diagonal w in (6,10) (validated), the
16 bins derive from R(c) = sum_j relu(w_ij - c) at c = 7, 8, 9 plus the
exact row sum T (host, fp64) via H[b] = R(b-1) - 2R(b) + R(b+1) with
linear fills (R(c) = T - cN for c <= 6) and zeros (c >= 10).

Device work per core (1024 rows of the label-sorted problem, 8 blocks
of 128): per 2048-col PSUM group, matmul (fp16 phi) then three
reduction taps read PSUM directly:
  ACT: relu((8-c) - p/8) + accum  -> R-part
  DVE: (p*0.125) min (8-c) + accum -> S-part, R-part = (8-c)*2048 - S
The same-class (xp) side uses a [128, win] band around the diagonal of
the sorted order: q = p/8 + A (A = 0 on own segment minus diagonal,
1000 elsewhere), then 3 DVE taps sum min(q, 8-c); R_p(c) = (8-c)*win - S.

Host (fp64): sort by label, T / Tp / diagonal handled exactly (no range
assumption on the diagonal), validation of the off-diagonal range from
R(7), R(9) against host expectations (fallback to numpy on violation),
second differences, entropies.
"""

import os
import numpy as np

import concourse.bass as bass
import concourse.mybir as mybir
import concourse.tile as tile
from concourse import bacc
from concourse.bass_utils import run_bass_kernel_spmd

N = 8192
NBIT = 64
NCORES = 8
ROWS_PER_CORE = N // NCORES          # 1024
BLOCKS = ROWS_PER_CORE // 128        # 8
NBINS = 16
EPS = 1e-7

GW = 2048                            # psum group width
NG = N // GW                         # 4 groups per block
TAPS = (7, 8, 9)                     # thresholds; a = 8 - c

# Engine per (tap_idx, group): 'A' = ACT (relu+accum), 'D' = DVE (min+accum).
# 7 ACT / 5 DVE; band (3 taps + q build) rides on DVE.
ASSIGN = {
    (0, 0): "A", (0, 1): "A", (0, 2): "A", (0, 3): "A",   # c=7
    (1, 0): "A", (1, 1): "D", (1, 2): "A", (1, 3): "D",   # c=8
    (2, 0): "D", (2, 1): "A", (2, 2): "D", (2, 3): "D",   # c=9
}

F32 = mybir.dt.float32
F16 = mybir.dt.float16

BIG = 1000.0                         # band mask offset for excluded columns

_PROGRAM_CACHE = {}


def _build_program(pad: int):
    win = 128 + 2 * pad              # band window width per 128-row block
    bw = ROWS_PER_CORE + 2 * pad     # per-core band width

    nc = bacc.Bacc(
        "TRN2", target_bir_lowering=False, debug=False, num_devices=NCORES
    )
    phiT_d = nc.dram_tensor("phiT", [NBIT, N], F16, kind="ExternalInput")
    bandT_d = nc.dram_tensor("bandT", [NBIT, bw], F16, kind="ExternalInput")
    amask_d = nc.dram_tensor("amask", [BLOCKS, 128, win], F16, kind="ExternalInput")
    # per block 16 f32 cols: [c7 g0..g3, c8 g0..g3, c9 g0..g3, band c7,c8,c9, 0]
    racc_d = nc.dram_tensor("racc", [BLOCKS, 128, 16], F32, kind="ExternalOutput")

    mn = mybir.AluOpType.min
    add = mybir.AluOpType.add
    mult = mybir.AluOpType.mult
    relu = mybir.ActivationFunctionType.Relu

    with tile.TileContext(nc) as tc:
        with (
            tc.tile_pool(name="const", bufs=1) as constp,
            tc.tile_pool(name="ascr", bufs=2) as ascrp,
            tc.tile_pool(name="dscr", bufs=2) as dscrp,
            tc.tile_pool(name="bq", bufs=2) as bqp,
            tc.tile_pool(name="acc", bufs=1) as accp,
            tc.tile_pool(name="ps", bufs=2, space=bass.MemorySpace.PSUM) as psp,
        ):
            phiT = constp.tile([NBIT, N], F16)
            bandT = constp.tile([NBIT, bw], F16)
            amask = constp.tile([128, BLOCKS * win], F16)
            nc.sync.dma_start(bandT[:], bandT_d[:])
            for g in range(NG):
                nc.sync.dma_start(
                    phiT[:, GW * g : GW * (g + 1)], phiT_d[:, GW * g : GW * (g + 1)]
                )
            for b in range(BLOCKS):
                nc.sync.dma_start(amask[:, b * win : (b + 1) * win], amask_d[b])

            racc_s = accp.tile([128, BLOCKS * 16], F32)
            nc.vector.memset(racc_s[:], 0.0)

            # per-tap ACT bias columns (const AP registry lacks -1.0)
            biases = constp.tile([128, len(TAPS)], F32)
            bias_col = {}
            for ti, c in enumerate(TAPS):
                nc.vector.memset(biases[:, ti : ti + 1], float(8 - c))
                bias_col[c] = biases[:, ti : ti + 1]

            # warm the ACT function table early (overlaps input DMA)
            warm = constp.tile([128, 1], F32)
            nc.vector.memset(warm[:], 0.0)
            wsc = constp.tile([128, 1], F32)
            nc.scalar.activation(wsc[:], warm[:], relu, bias=bias_col[8], scale=1.0)

            for blk in range(BLOCKS):
                own = bandT[:, pad + 128 * blk : pad + 128 * (blk + 1)]
                r0 = blk * 16

                def full_group(g, r0=r0, own=own):
                    pp = psp.tile([128, GW], F32, tag="pp")
                    for s in range(GW // 512):
                        nc.tensor.matmul(
                            pp[:, 512 * s : 512 * (s + 1)],
                            own,
                            phiT[:, GW * g + 512 * s : GW * g + 512 * (s + 1)],
                            start=True,
                            stop=True,
                        )
                    for ti, c in enumerate(TAPS):
                        a = float(8 - c)
                        col = r0 + 4 * ti + g
                        if ASSIGN[(ti, g)] == "A":
                            scr = ascrp.tile([128, GW], F32, tag="as")
                            nc.scalar.activation(
                                scr[:], pp[:], relu, bias=bias_col[c], scale=-0.125,
                                accum_out=racc_s[:, col : col + 1],
                            )
                        else:
                            # out = min(pp, 8a); accum = sum  -> 8*sum(min(t,a))
                            scr = dscrp.tile([128, GW], F32, tag="ds")
                            nc.vector.tensor_scalar(
                                scr[:], pp[:], 8.0 * a, None, mn, add,
                                accum_out=racc_s[:, col : col + 1],
                            )

                def band_work(blk=blk, r0=r0, own=own):
                    # band: matmul window into a rotation slot, mask-add, 3 taps
                    ppb = psp.tile([128, GW], F32, tag="pp")
                    off = 0
                    while off < win:
                        cw = min(512, win - off)
                        nc.tensor.matmul(
                            ppb[:, off : off + cw],
                            own,
                            bandT[:, 128 * blk + off : 128 * blk + off + cw],
                            start=True,
                            stop=True,
                        )
                        off += cw
                    q = bqp.tile([128, win], F32, tag="q")
                    nc.vector.scalar_tensor_tensor(
                        q[:], ppb[:, 0:win], 0.125,
                        amask[:, blk * win : (blk + 1) * win], mult, add,
                    )
                    for ti, c in enumerate(TAPS):
                        a = float(8 - c)
                        scr = bqp.tile([128, win], F32, tag="bs")
                        nc.vector.tensor_scalar(
                            scr[:], q[:], a, None, mn, add,
                            accum_out=racc_s[:, r0 + 12 + ti : r0 + 13 + ti],
                        )

                # band in the middle: next block's first matmul then overlaps
                # the last group's taps instead of waiting on the band tail
                full_group(0)
                full_group(1)
                band_work()
                full_group(2)
                full_group(3)

            for blk in range(BLOCKS):
                nc.sync.dma_start(racc_d[blk], racc_s[:, blk * 16 : (blk + 1) * 16])

    nc.compile()
    return nc, win, bw


class _RangeViolation(Exception):
    pass


def _numpy_reference(u, y):
    """Exact fp64 fallback (non-one-hot y or off-diagonal range violation)."""
    u = u.astype(np.float64)
    y = y.astype(np.float64)
    n, nbits = u.shape
    aff = ((y @ y.T) > 0).astype(np.float64)
    np.fill_diagonal(aff, 0.0)
    xp = aff
    xn = 1.0 - aff
    phi = 2.0 / (1.0 + np.exp(-u)) - 1.0
    dist = (nbits - phi @ phi.T) * 0.5
    prCp = xp.sum(1) / (n - 1)
    prCn = 1.0 - prCp
    delta = nbits // NBINS
    pDCp = np.zeros((n, NBINS))
    pDCn = np.zeros((n, NBINS))
    for b in range(NBINS):
        mid = b * delta
        ind = (dist > mid - delta) & (dist <= mid + delta)
        pulse = np.where(ind, 1.0 - np.abs(dist - mid) / delta, 0.0)
        pDCp[:, b] = (pulse * xp).sum(1)
        pDCn[:, b] = (pulse * xn).sum(1)
    return _finish_loss(pDCp, pDCn, prCp, prCn, n)


def _finish_loss(pDCp, pDCn, prCp, prCn, n):
    pD = (pDCp + pDCn) / (n - 1)
    sum_p = pDCp.sum(1)
    sum_n = pDCn.sum(1)
    safe_p = np.where(sum_p > 0, sum_p, 1.0)
    safe_n = np.where(sum_n > 0, sum_n, 1.0)
    pDCp = np.where((sum_p > 0)[:, None], pDCp / safe_p[:, None], pDCp)
    pDCn = np.where((sum_n > 0)[:, None], pDCn / safe_n[:, None], pDCn)

    def ent(p):
        return -(p * np.log(p + EPS)).sum(1)

    loss = (ent(pD) - (prCp * ent(pDCp) + prCn * ent(pDCn))).sum()
    return np.array(loss, dtype=np.float32)


def _hat(x):
    return np.maximum(0.0, 1.0 - np.abs(x))


_LAST_RESULTS = None


def kernel(u, y):
    u = np.ascontiguousarray(np.asarray(u), dtype=np.float32)
    y = np.asarray(y)
    assert u.shape == (N, NBIT)

    pos = y > 0
    if not (pos.sum(axis=1) == 1).all() or (y < 0).any():
        return _numpy_reference(u, np.asarray(y, np.float32))
    labels = pos.argmax(axis=1)

    perm = np.argsort(labels, kind="stable")
    labels_s = labels[perm]
    counts = np.bincount(labels_s, minlength=labels_s.max() + 1)
    starts = np.concatenate([[0], np.cumsum(counts)])
    seg_s = starts[labels_s]
    seg_e = starts[labels_s + 1]
    maxn = int(counts.max())

    pad = 128
    while maxn - 1 > pad:
        pad += 128
    win = 128 + 2 * pad
    bw = ROWS_PER_CORE + 2 * pad

    if pad not in _PROGRAM_CACHE:
        _PROGRAM_CACHE[pad] = _build_program(pad)
    nc, win_, bw_ = _PROGRAM_CACHE[pad]
    assert (win_, bw_) == (win, bw)

    phi16 = np.tanh(u / 2.0).astype(np.float16)
    phiT = np.ascontiguousarray(phi16[perm].T)           # [64, N] f16, sorted
    phi64 = phiT.T.astype(np.float64)

    # host-exact sums (same f16 phi values the device sees)
    s_all = phi64.sum(axis=0)
    t_row = (phi64 @ s_all) / 8.0                        # sum_j t_ij incl diag
    t_diag = (phi64 * phi64).sum(axis=1) / 8.0
    w_diag = 8.0 - t_diag
    T_all = 8.0 * N - t_row                              # sum_j w_ij incl diag

    ncls = len(counts)
    cls_sums = np.zeros((ncls, NBIT))
    np.add.at(cls_sums, labels_s, phi64)
    nseg = (seg_e - seg_s).astype(np.float64)            # class size incl self
    Sp = nseg - 1.0
    Tp = 8.0 * Sp - ((phi64 * (cls_sums[labels_s] - phi64)).sum(axis=1)) / 8.0

    in_maps = []
    for core in range(NCORES):
        off = core * ROWS_PER_CORE
        lo = off - pad
        band = np.zeros((NBIT, bw), dtype=np.float16)
        c0 = max(0, lo)
        c1 = min(N, off + ROWS_PER_CORE + pad)
        band[:, c0 - lo : c1 - lo] = phiT[:, c0:c1]

        am = np.full((BLOCKS, 128, win), BIG, dtype=np.float16)
        idx = np.arange(win)[None, :]
        for blk in range(BLOCKS):
            w0 = off + 128 * blk - pad                   # global col of window x=0
            rows = np.arange(off + 128 * blk, off + 128 * (blk + 1))
            xs = seg_s[rows] - w0
            xe = seg_e[rows] - w0
            assert (xs >= 0).all() and (xe <= win).all(), "segment outside window"
            inside = (idx >= xs[:, None]) & (idx < xe[:, None])
            am[blk][inside] = 0.0
            am[blk, np.arange(128), rows - w0] = BIG     # exclude diagonal
        in_maps.append({"phiT": phiT, "bandT": band, "amask": am})

    try:
        return _postprocess_and_loss(
            nc, in_maps, seg_s, seg_e, pad, T_all, Tp, Sp, w_diag
        )
    except _RangeViolation:
        return _numpy_reference(u, np.asarray(y, np.float32))


def _postprocess_and_loss(nc, in_maps, seg_s, seg_e, pad, T_all, Tp, Sp, w_diag):
    global _LAST_RESULTS
    res = run_bass_kernel_spmd(nc, in_maps, list(range(NCORES)))
    _LAST_RESULTS = res

    win = 128 + 2 * pad
    pDCp = np.zeros((N, NBINS))
    pDCn = np.zeros((N, NBINS))
    for core in range(NCORES):
        out = res.results[core]
        racc = out["racc"].astype(np.float64)            # [8, 128, 16]
        off = core * ROWS_PER_CORE
        rows = np.arange(off, off + ROWS_PER_CORE)

        # full side: R_all(c) from mixed-engine group parts
        R_all = np.zeros((ROWS_PER_CORE, 3))
        for ti, c in enumerate(TAPS):
            a = float(8 - c)
            acc = np.zeros(ROWS_PER_CORE)
            for g in range(NG):
                part = racc[:, :, 4 * ti + g].reshape(-1)
                if ASSIGN[(ti, g)] == "A":
                    acc += part                          # relu sums directly
                else:
                    acc += a * GW - part / 8.0           # (8-c)*GW - sum(min(t,a))
            R_all[:, ti] = acc

        wd = w_diag[rows]
        Td = T_all[rows]
        # validation: off-diagonal w must lie in (6, 10)
        L7 = R_all[:, 0] + 7.0 * N - Td                  # sum relu(7-w) incl diag
        exc7 = L7 - np.maximum(7.0 - wd, 0.0)
        R9p = R_all[:, 2] - np.maximum(wd - 9.0, 0.0)    # off-diag sum relu(w-9)
        if (exc7 > 0.5).any() or (R9p > 0.5).any() or (exc7 < -0.5).any():
            raise _RangeViolation()

        # off-diagonal R'(c), linear fills, second differences
        Rp_ = np.zeros((ROWS_PER_CORE, 18))              # c = -1 .. 16
        Tdp = Td - wd                                    # off-diag row sum
        for c in range(-1, 7):
            Rp_[:, c + 1] = Tdp - float(c) * (N - 1)
        for ti, c in enumerate(TAPS):
            Rp_[:, c + 1] = R_all[:, ti] - np.maximum(wd - c, 0.0)
        H_all = Rp_[:, 0:16] - 2.0 * Rp_[:, 1:17] + Rp_[:, 2:18]
        H_all[:, :6] = 0.0
        H_all[:, 11:] = 0.0
        H_all = np.maximum(H_all, 0.0)

        # band side: R_p(c) = (8-c)*win - S_band
        Rb = np.zeros((ROWS_PER_CORE, 18))
        Spr = Sp[rows]
        Tpr = Tp[rows]
        for c in range(-1, 7):
            Rb[:, c + 1] = Tpr - float(c) * Spr
        for ti, c in enumerate(TAPS):
            a = float(8 - c)
            Rb[:, c + 1] = a * win - racc[:, :, 12 + ti].reshape(-1)
        H_p = Rb[:, 0:16] - 2.0 * Rb[:, 1:17] + Rb[:, 2:18]
        H_p[:, :6] = 0.0
        H_p[:, 11:] = 0.0
        H_p = np.maximum(H_p, 0.0)

        H_n = np.maximum(H_all - H_p, 0.0)
        # diagonal contributes to the xn histogram (xn_ii = 1)
        bins = np.arange(NBINS)[None, :]
        H_n += _hat(wd[:, None] - bins)
        pDCp[rows] = H_p
        pDCn[rows] = H_n

    prCp = Sp / (N - 1)
    prCn = 1.0 - prCp
    return _finish_loss(pDCp, pDCn, prCp, prCn, N)


# revision 12
# speedup vs baseline: 2.9150x; 1.2233x over previous
"""MIHash loss kernel for Trainium2 (8 NeuronCores, SPMD) — v3.

Math: loss = sum_i ent(pD_i) - prCp_i*ent(pDCp_i) - prCn_i*ent(pDCn_i),
histograms from triangular pulses of w = dist/4 = 8 - phi_i.phi_j/8,
hat(w-b) for b = 0..15.  With off-diagonal w in (6,10) (validated), the
16 bins derive from R(c) = sum_j relu(w_ij - c) at c = 7, 8, 9 plus the
exact row sum T (host, fp64) via H[b] = R(b-1) - 2R(b) + R(b+1) with
linear fills (R(c) = T - cN for c <= 6) and zeros (c >= 10).

Device (per core: 1024 rows of the label-sorted problem, 8 blocks of
128): the 8192 columns are processed in 4 supergroups of 2048, each
split into two 1024-col PSUM tiles — one consumed by ACT, one by DVE.
Cross-engine reads of one PSUM tile serialize on TRN2, so each engine
owns private tiles and the two reduction pipelines run independently:
  ACT (left half):  relu((8-c) - p/8) + accum            -> R-part
  DVE (right half): min(p, 8(8-c)) + accum, /8 on host   -> S-part
The same-class (xp) side uses a [128, win] band around the diagonal of
the sorted order (window from a per-core bandT input): q = p/8 + A
(A = 0 on own segment minus diagonal, 1000 elsewhere), 3 DVE taps sum
min(q, 8-c); R_p(c) = (8-c)*win - S.

Host (fp64): sort by label, T / Tp / diagonal handled exactly (no range
assumption on the diagonal), validation of the off-diagonal range from
R(7), R(9) against host expectations (fallback to numpy on violation),
second differences, entropies.
"""

import numpy as np

import concourse.bass as bass
import concourse.mybir as mybir
import concourse.tile as tile
from concourse import bacc
from concourse.bass_utils import run_bass_kernel_spmd

N = 8192
NBIT = 64
NCORES = 8
ROWS_PER_CORE = N // NCORES          # 1024
BLOCKS = ROWS_PER_CORE // 128        # 8
NBINS = 16
EPS = 1e-7

SG = 2048                            # supergroup width
NSG = N // SG                        # 4 per block
HW_ = SG // 2                        # 1024 cols per engine per supergroup
TAPS = (7, 8, 9)

F32 = mybir.dt.float32
F16 = mybir.dt.float16

BIG = 1000.0                         # band mask offset for excluded columns

_PROGRAM_CACHE = {}

# racc column layout per block (32 cols):
#   ti*8 + sg*2 + 0 : ACT relu-sum, cols [sg*2048, +1024)
#   ti*8 + sg*2 + 1 : DVE min-sum*8, cols [sg*2048+1024, +1024)
#   24+ti           : band min-sum over q
NCOL = 32


def _build_program(pad: int):
    win = 128 + 2 * pad
    bw = ROWS_PER_CORE + 2 * pad

    nc = bacc.Bacc(
        "TRN2", target_bir_lowering=False, debug=False, num_devices=NCORES
    )
    phiT_d = nc.dram_tensor("phiT", [NBIT, N], F16, kind="ExternalInput")
    bandT_d = nc.dram_tensor("bandT", [NBIT, bw], F16, kind="ExternalInput")
    amask_d = nc.dram_tensor("amask", [BLOCKS, 128, win], F16, kind="ExternalInput")
    racc_d = nc.dram_tensor("racc", [BLOCKS, 128, NCOL], F32, kind="ExternalOutput")

    mn = mybir.AluOpType.min
    add = mybir.AluOpType.add
    mult = mybir.AluOpType.mult
    relu = mybir.ActivationFunctionType.Relu

    with tile.TileContext(nc) as tc:
        with (
            tc.tile_pool(name="const", bufs=1) as constp,
            tc.tile_pool(name="ascr", bufs=2) as ascrp,
            tc.tile_pool(name="dscr", bufs=2) as dscrp,
            tc.tile_pool(name="bq", bufs=2) as bqp,
            tc.tile_pool(name="acc", bufs=1) as accp,
            tc.tile_pool(name="psA", bufs=2, space=bass.MemorySpace.PSUM) as psA,
            tc.tile_pool(name="psD", bufs=2, space=bass.MemorySpace.PSUM) as psD,
        ):
            phiT = constp.tile([NBIT, N], F16)
            bandT = constp.tile([NBIT, bw], F16)
            amask = constp.tile([128, BLOCKS * win], F16)
            nc.sync.dma_start(bandT[:], bandT_d[:])
            for g in range(NSG):
                nc.sync.dma_start(
                    phiT[:, SG * g : SG * (g + 1)], phiT_d[:, SG * g : SG * (g + 1)]
                )
            for b in range(BLOCKS):
                nc.sync.dma_start(amask[:, b * win : (b + 1) * win], amask_d[b])

            racc_s = accp.tile([128, BLOCKS * NCOL], F32)
            nc.vector.memset(racc_s[:], 0.0)

            biases = constp.tile([128, len(TAPS)], F32)
            bias_col = {}
            for ti, c in enumerate(TAPS):
                nc.vector.memset(biases[:, ti : ti + 1], float(8 - c))
                bias_col[c] = biases[:, ti : ti + 1]

            # warm the ACT function table early (overlaps input DMA)
            warm = constp.tile([128, 1], F32)
            nc.vector.memset(warm[:], 0.0)
            wsc = constp.tile([128, 1], F32)
            nc.scalar.activation(wsc[:], warm[:], relu, bias=bias_col[8], scale=1.0)

            for blk in range(BLOCKS):
                own = bandT[:, pad + 128 * blk : pad + 128 * (blk + 1)]
                r0 = blk * NCOL

                def supergroup(sg, r0=r0, own=own, blk=blk):
                    base = SG * sg
                    ppa = psA.tile([128, HW_], F32, tag="ppA")
                    ppd = psD.tile([128, HW_], F32, tag="ppD")
                    for s in range(HW_ // 512):
                        nc.tensor.matmul(
                            ppa[:, 512 * s : 512 * (s + 1)],
                            own,
                            phiT[:, base + 512 * s : base + 512 * (s + 1)],
                            start=True,
                            stop=True,
                        ).annotate(f"mmA_b{blk}s{sg}")
                    for s in range(HW_ // 512):
                        nc.tensor.matmul(
                            ppd[:, 512 * s : 512 * (s + 1)],
                            own,
                            phiT[:, base + HW_ + 512 * s : base + HW_ + 512 * (s + 1)],
                            start=True,
                            stop=True,
                        ).annotate(f"mmD_b{blk}s{sg}")
                    for ti, c in enumerate(TAPS):
                        a = float(8 - c)
                        scr = ascrp.tile([128, HW_], F32, tag="as")
                        nc.scalar.activation(
                            scr[:], ppa[:], relu, bias=bias_col[c], scale=-0.125,
                            accum_out=racc_s[:, r0 + 8 * ti + 2 * sg : r0 + 8 * ti + 2 * sg + 1],
                        ).annotate(f"tapA_b{blk}s{sg}c{c}")
                        scr2 = dscrp.tile([128, HW_], F32, tag="ds")
                        nc.vector.tensor_scalar(
                            scr2[:], ppd[:], 8.0 * a, None, mn, add,
                            accum_out=racc_s[:, r0 + 8 * ti + 2 * sg + 1 : r0 + 8 * ti + 2 * sg + 2],
                        ).annotate(f"tapD_b{blk}s{sg}c{c}")

                def band_work(blk=blk, r0=r0, own=own):
                    ppb = psD.tile([128, HW_], F32, tag="ppD")
                    off = 0
                    while off < win:
                        cw = min(512, win - off)
                        nc.tensor.matmul(
                            ppb[:, off : off + cw],
                            own,
                            bandT[:, 128 * blk + off : 128 * blk + off + cw],
                            start=True,
                            stop=True,
                        ).annotate(f"mmband_b{blk}")
                        off += cw
                    q = bqp.tile([128, win], F32, tag="q")
                    nc.vector.scalar_tensor_tensor(
                        q[:], ppb[:, 0:win], 0.125,
                        amask[:, blk * win : (blk + 1) * win], mult, add,
                    ).annotate(f"qbuild_b{blk}")
                    for ti, c in enumerate(TAPS):
                        a = float(8 - c)
                        scr = bqp.tile([128, win], F32, tag="bs")
                        nc.vector.tensor_scalar(
                            scr[:], q[:], a, None, mn, add,
                            accum_out=racc_s[:, r0 + 24 + ti : r0 + 25 + ti],
                        ).annotate(f"tapB_b{blk}c{c}")

                supergroup(0)
                supergroup(1)
                band_work()
                supergroup(2)
                supergroup(3)

            for blk in range(BLOCKS):
                nc.sync.dma_start(
                    racc_d[blk], racc_s[:, blk * NCOL : (blk + 1) * NCOL]
                )

    nc.compile()
    return nc, win, bw


class _RangeViolation(Exception):
    pass


def _numpy_reference(u, y):
    """Exact fp64 fallback (non-one-hot y or off-diagonal range violation)."""
    u = u.astype(np.float64)
    y = y.astype(np.float64)
    n, nbits = u.shape
    aff = ((y @ y.T) > 0).astype(np.float64)
    np.fill_diagonal(aff, 0.0)
    xp = aff
    xn = 1.0 - aff
    phi = 2.0 / (1.0 + np.exp(-u)) - 1.0
    dist = (nbits - phi @ phi.T) * 0.5
    prCp = xp.sum(1) / (n - 1)
    prCn = 1.0 - prCp
    delta = nbits // NBINS
    pDCp = np.zeros((n, NBINS))
    pDCn = np.zeros((n, NBINS))
    for b in range(NBINS):
        mid = b * delta
        ind = (dist > mid - delta) & (dist <= mid + delta)
        pulse = np.where(ind, 1.0 - np.abs(dist - mid) / delta, 0.0)
        pDCp[:, b] = (pulse * xp).sum(1)
        pDCn[:, b] = (pulse * xn).sum(1)
    return _finish_loss(pDCp, pDCn, prCp, prCn, n)


def _finish_loss(pDCp, pDCn, prCp, prCn, n):
    pD = (pDCp + pDCn) / (n - 1)
    sum_p = pDCp.sum(1)
    sum_n = pDCn.sum(1)
    safe_p = np.where(sum_p > 0, sum_p, 1.0)
    safe_n = np.where(sum_n > 0, sum_n, 1.0)
    pDCp = np.where((sum_p > 0)[:, None], pDCp / safe_p[:, None], pDCp)
    pDCn = np.where((sum_n > 0)[:, None], pDCn / safe_n[:, None], pDCn)

    def ent(p):
        return -(p * np.log(p + EPS)).sum(1)

    loss = (ent(pD) - (prCp * ent(pDCp) + prCn * ent(pDCn))).sum()
    return np.array(loss, dtype=np.float32)


def _hat(x):
    return np.maximum(0.0, 1.0 - np.abs(x))


_LAST_RESULTS = None


def kernel(u, y):
    u = np.ascontiguousarray(np.asarray(u), dtype=np.float32)
    y = np.asarray(y)
    assert u.shape == (N, NBIT)

    pos = y > 0
    if not (pos.sum(axis=1) == 1).all() or (y < 0).any():
        return _numpy_reference(u, np.asarray(y, np.float32))
    labels = pos.argmax(axis=1)

    perm = np.argsort(labels, kind="stable")
    labels_s = labels[perm]
    counts = np.bincount(labels_s, minlength=labels_s.max() + 1)
    starts = np.concatenate([[0], np.cumsum(counts)])
    seg_s = starts[labels_s]
    seg_e = starts[labels_s + 1]
    maxn = int(counts.max())

    pad = 128
    while maxn - 1 > pad:
        pad += 128
    win = 128 + 2 * pad
    bw = ROWS_PER_CORE + 2 * pad

    if pad not in _PROGRAM_CACHE:
        _PROGRAM_CACHE[pad] = _build_program(pad)
    nc, win_, bw_ = _PROGRAM_CACHE[pad]
    assert (win_, bw_) == (win, bw)

    phi16 = np.tanh(u / 2.0).astype(np.float16)
    phiT = np.ascontiguousarray(phi16[perm].T)           # [64, N] f16, sorted
    phi64 = phiT.T.astype(np.float64)

    s_all = phi64.sum(axis=0)
    t_row = (phi64 @ s_all) / 8.0                        # sum_j t_ij incl diag
    t_diag = (phi64 * phi64).sum(axis=1) / 8.0
    w_diag = 8.0 - t_diag
    T_all = 8.0 * N - t_row

    ncls = len(counts)
    cls_sums = np.zeros((ncls, NBIT))
    np.add.at(cls_sums, labels_s, phi64)
    nseg = (seg_e - seg_s).astype(np.float64)
    Sp = nseg - 1.0
    Tp = 8.0 * Sp - ((phi64 * (cls_sums[labels_s] - phi64)).sum(axis=1)) / 8.0

    in_maps = []
    for core in range(NCORES):
        off = core * ROWS_PER_CORE
        lo = off - pad
        band = np.zeros((NBIT, bw), dtype=np.float16)
        c0 = max(0, lo)
        c1 = min(N, off + ROWS_PER_CORE + pad)
        band[:, c0 - lo : c1 - lo] = phiT[:, c0:c1]

        am = np.full((BLOCKS, 128, win), BIG, dtype=np.float16)
        idx = np.arange(win)[None, :]
        for blk in range(BLOCKS):
            w0 = off + 128 * blk - pad
            rows = np.arange(off + 128 * blk, off + 128 * (blk + 1))
            xs = seg_s[rows] - w0
            xe = seg_e[rows] - w0
            assert (xs >= 0).all() and (xe <= win).all(), "segment outside window"
            inside = (idx >= xs[:, None]) & (idx < xe[:, None])
            am[blk][inside] = 0.0
            am[blk, np.arange(128), rows - w0] = BIG     # exclude diagonal
        in_maps.append({"phiT": phiT, "bandT": band, "amask": am})

    try:
        return _postprocess_and_loss(
            nc, in_maps, seg_s, seg_e, pad, T_all, Tp, Sp, w_diag
        )
    except _RangeViolation:
        return _numpy_reference(u, np.asarray(y, np.float32))


def _postprocess_and_loss(nc, in_maps, seg_s, seg_e, pad, T_all, Tp, Sp, w_diag):
    global _LAST_RESULTS
    res = run_bass_kernel_spmd(nc, in_maps, list(range(NCORES)))
    _LAST_RESULTS = res

    win = 128 + 2 * pad
    pDCp = np.zeros((N, NBINS))
    pDCn = np.zeros((N, NBINS))
    for core in range(NCORES):
        out = res.results[core]
        racc = out["racc"].astype(np.float64)            # [8, 128, 32]
        off = core * ROWS_PER_CORE
        rows = np.arange(off, off + ROWS_PER_CORE)

        R_all = np.zeros((ROWS_PER_CORE, 3))
        for ti, c in enumerate(TAPS):
            a = float(8 - c)
            acc = np.zeros(ROWS_PER_CORE)
            for sg in range(NSG):
                acc += racc[:, :, 8 * ti + 2 * sg].reshape(-1)          # ACT
                acc += a * HW_ - racc[:, :, 8 * ti + 2 * sg + 1].reshape(-1) / 8.0
            R_all[:, ti] = acc

        wd = w_diag[rows]
        Td = T_all[rows]
        L7 = R_all[:, 0] + 7.0 * N - Td                  # sum relu(7-w) incl diag
        exc7 = L7 - np.maximum(7.0 - wd, 0.0)
        R9p = R_all[:, 2] - np.maximum(wd - 9.0, 0.0)
        if (exc7 > 0.5).any() or (R9p > 0.5).any() or (exc7 < -0.5).any():
            raise _RangeViolation()

        Rt = np.zeros((ROWS_PER_CORE, 18))               # c = -1 .. 16
        Tdp = Td - wd
        for c in range(-1, 7):
            Rt[:, c + 1] = Tdp - float(c) * (N - 1)
        for ti, c in enumerate(TAPS):
            Rt[:, c + 1] = R_all[:, ti] - np.maximum(wd - c, 0.0)
        H_all = Rt[:, 0:16] - 2.0 * Rt[:, 1:17] + Rt[:, 2:18]
        H_all[:, :6] = 0.0
        H_all[:, 11:] = 0.0
        H_all = np.maximum(H_all, 0.0)

        Rb = np.zeros((ROWS_PER_CORE, 18))
        Spr = Sp[rows]
        Tpr = Tp[rows]
        for c in range(-1, 7):
            Rb[:, c + 1] = Tpr - float(c) * Spr
        for ti, c in enumerate(TAPS):
            a = float(8 - c)
            Rb[:, c + 1] = a * win - racc[:, :, 24 + ti].reshape(-1)
        H_p = Rb[:, 0:16] - 2.0 * Rb[:, 1:17] + Rb[:, 2:18]
        H_p[:, :6] = 0.0
        H_p[:, 11:] = 0.0
        H_p = np.maximum(H_p, 0.0)

        H_n = np.maximum(H_all - H_p, 0.0)
        bins = np.arange(NBINS)[None, :]
        H_n += _hat(wd[:, None] - bins)                  # diagonal (xn_ii = 1)
        pDCp[rows] = H_p
        pDCn[rows] = H_n

    prCp = Sp / (N - 1)
    prCn = 1.0 - prCp
    return _finish_loss(pDCp, pDCn, prCp, prCn, N)
